# revision 18
# baseline (speedup 1.0000x reference)
"""Trainium2 Bass kernel for the O2O classification head (GNN message passing).

Strategy (v2)
-------------
The edge tensor is rank-structured: after the first edge-MLP layer the
pre-gelu value for pair (i, j) is u = p_i + qneg_j with per-node vectors
    p    = (feats@W_in + pos@W_pos + b_in + b_pos) @ W_e1
    qneg = b_e1 - (feats@W_out + pos@W_pos + b_out) @ W_e1
so the device only computes gelu(p_i + qneg_j) . W_e2 per pair.

Host-side, nodes are sorted by (cls desc, id desc); suppress[i,j] != 0
requires rank_i < rank_j.  Ranks with cls < 0.4 have output exactly
sigmoid(-1e6) = 0, and sorting puts them at ranks >= K, so only the
top-K block is computed at all (K = #{cls >= 0.4} rounded up to 16).

Sharding: 2 cores per batch; core parity P takes ranks == P (mod 2).
Per core, j's are processed in t-blocks of 8 with i-prefix 16(t+1).
All pair work is bf16: DVE broadcasts u = p + qneg (4x mode), Act does
gelu, PE dot-products with W_e2 accumulate into PSUM on top of a
host-precomputed additive mask (0 where allowed, -30000 elsewhere,
b_e2 folded in) injected via an identity-stationary matmul.  A row-max
per t-block then gives node_max; the tiny 64-wide output MLP and the
sigmoid run on host.
"""

import sys
import numpy as np
import ml_dtypes

if "/opt/trn_rl_repo" not in sys.path:
    sys.path.insert(0, "/opt/trn_rl_repo")

B, N = 4, 512
H_DIM, I_DIM = 64, 128
N_CORES = 8
TJ = 8               # j's per t-block
GROUP_W = 512        # max PSUM bank width (fp32 cols)
F32 = np.float32
BF16 = ml_dtypes.bfloat16

IMG_W, IMG_H, CENTER_H = 800.0, 320.0, 160.0
NUM_OFFSETS = 72
CONF_THRES = 0.4
MASK_NEG = -30000.0

_PROGRAMS = {}       # n_t -> compiled program
_LAST_NT = None


def _lens(n_t):
    return [16 * (t + 1) for t in range(n_t)]


def _groups(n_t):
    """Greedy grouping of consecutive t-blocks with sum(L) <= GROUP_W."""
    Ls = _lens(n_t)
    groups, cur, cur_w = [], [], 0
    for t in range(n_t):
        if cur and cur_w + Ls[t] > GROUP_W:
            groups.append((cur, cur_w))
            cur, cur_w = [], 0
        cur.append(t)
        cur_w += Ls[t]
    groups.append((cur, cur_w))
    return groups


def _build_program(n_t, num_devices=N_CORES):
    import contextlib
    import concourse.bass as bass  # noqa: F401
    import concourse.tile as tile
    from concourse import bacc, mybir

    f32 = mybir.dt.float32
    bf16 = mybir.dt.bfloat16
    AF = mybir.ActivationFunctionType
    AX = mybir.AxisListType

    Ls = _lens(n_t)
    groups = _groups(n_t)
    Lsum = sum(Ls)
    J = TJ * n_t
    K = 2 * J

    nc = bacc.Bacc("TRN2", target_bir_lowering=False, debug=False,
                   num_devices=num_devices)

    d_p16 = nc.declare_dram_parameter("p16", [I_DIM, K], bf16, isOutput=False)
    d_qneg = nc.declare_dram_parameter("qneg", [I_DIM, J], f32, isOutput=False)
    d_madd = nc.declare_dram_parameter("madd", [TJ, Lsum], bf16,
                                       isOutput=False)
    d_we2 = nc.declare_dram_parameter("we2", [I_DIM, TJ * TJ], bf16,
                                      isOutput=False)
    d_i8 = nc.declare_dram_parameter("i8", [TJ, TJ], bf16, isOutput=False)
    y = nc.declare_dram_parameter("y", [TJ, n_t], f32, isOutput=True)

    with tile.TileContext(nc) as tc:
        with contextlib.ExitStack() as ctx:
            const = ctx.enter_context(tc.tile_pool(name="const", bufs=1))
            ub = ctx.enter_context(tc.tile_pool(name="ub", bufs=2))
            gb = ctx.enter_context(tc.tile_pool(name="gb", bufs=2))
            sp = ctx.enter_context(tc.tile_pool(name="sp", bufs=3,
                                                space="PSUM"))

            p16 = const.tile([I_DIM, K], bf16, name="p16", tag="p16")
            qneg = const.tile([I_DIM, J], f32, name="qneg", tag="qneg")
            madd = const.tile([TJ, Lsum], bf16, name="madd", tag="madd")
            we2 = const.tile([I_DIM, TJ * TJ], bf16, name="we2", tag="we2")
            i8 = const.tile([TJ, TJ], bf16, name="i8", tag="i8")
            nc.gpsimd.dma_start(out=p16[:], in_=d_p16[:])
            nc.gpsimd.dma_start(out=qneg[:], in_=d_qneg[:])
            nc.gpsimd.dma_start(out=madd[:], in_=d_madd[:])
            nc.gpsimd.dma_start(out=we2[:], in_=d_we2[:])
            nc.gpsimd.dma_start(out=i8[:], in_=d_i8[:])

            nmall = const.tile([TJ, n_t], f32, name="nmall", tag="nmall")

            UW = TJ * GROUP_W
            pending = []

            def flush_one():
                S, ts, offs, w = pending.pop(0)
                for t, off in zip(ts, offs):
                    nc.vector.reduce_max(nmall[:, t:t + 1],
                                         S[:, off:off + Ls[t]], axis=AX.X)

            # Per-j U-adds carry a ~225ns fixed cost each on DVE (the 4x-mode
            # exec itself is ~0.18 ns/col).  Split the J per-j instructions
            # between DVE and the mostly-idle GPSIMD engine, greedily keeping
            # the running engine loads (in ns, measured costs) balanced.
            dve_load = sum(136 + 1.06 * L for L in Ls)   # reduces stay on DVE
            gps_load = 7300.0                            # input DMA issue
            on_gps = set()
            for t in range(n_t):
                L = Ls[t]
                d_c, g_c = 225 + 0.182 * L, 245 + 1.39 * L
                for jj in range(TJ):
                    if gps_load + g_c < dve_load + d_c:
                        on_gps.add((t, jj))
                        gps_load += g_c
                    else:
                        dve_load += d_c

            goff = 0
            for gi, (ts, w) in enumerate(groups):
                U = ub.tile([I_DIM, UW], bf16, name=f"U{gi}", tag="U")
                G = gb.tile([I_DIM, UW], bf16, name=f"G{gi}", tag="G")
                off = 0
                offs = []
                for t in ts:
                    L = Ls[t]
                    for jj in range(TJ):
                        eng = nc.gpsimd if (t, jj) in on_gps else nc.vector
                        eng.tensor_scalar_add(
                            U[:, jj * w + off: jj * w + off + L],
                            p16[:, :L],
                            qneg[:, TJ * t + jj: TJ * t + jj + 1])
                    offs.append(off)
                    off += L
                nc.scalar.activation(G[:, :TJ * w], U[:, :TJ * w], AF.Gelu)

                S = sp.tile([TJ, GROUP_W], f32, name=f"S{gi}", tag="S")
                nc.tensor.matmul(S[:, :w], i8[:], madd[:, goff:goff + w],
                                 start=True, stop=False)
                for jj in range(TJ):
                    nc.tensor.matmul(S[:, :w], we2[:, TJ * jj: TJ * (jj + 1)],
                                     G[:, jj * w: (jj + 1) * w],
                                     start=False, stop=(jj == TJ - 1))
                pending.append((S, ts, offs, w))
                goff += w
                if len(pending) > 1:
                    flush_one()

            while pending:
                flush_one()

            nc.gpsimd.dma_start(out=y[:], in_=nmall[:])

    nc.compile()
    return nc


def _get_program(n_t=None):
    global _LAST_NT
    if n_t is None:
        n_t = _LAST_NT
    if n_t not in _PROGRAMS:
        _PROGRAMS[n_t] = _build_program(n_t)
    _LAST_NT = n_t
    return _PROGRAMS[n_t]


def _pos_emb(e0, e1):
    """float32 mirror of the reference _get_sample_point (one batch, sorted)."""
    angle = (e0 * F32(np.pi)).astype(F32)
    rho = (e1 * F32(IMG_W)).astype(F32)
    lin = np.linspace(0.0, 1.0 - 1e-5, NUM_OFFSETS, dtype=F32)
    yk = (F32(CENTER_H) - lin * F32(IMG_H)).astype(F32)[:2]
    tan = np.tan(angle, dtype=F32)
    roc = (rho / np.cos(angle, dtype=F32)).astype(F32)
    x = (-tan[:, None] * yk[None, :] + roc[:, None]).astype(F32)
    return (x / F32(IMG_W)).astype(F32)          # [n, 2]


def kernel(**inputs):
    global _LAST_NT
    bf = np.asarray(inputs["batch_features"], dtype=F32)      # [B,N,64]
    cls = np.asarray(inputs["cls_pred"], dtype=F32)           # [B,N]
    aid = np.asarray(inputs["anchor_id"])                     # [B,N] int32
    emb = np.asarray(inputs["anchor_embeddings"], dtype=F32)  # [B,N,2]

    w = {k: np.asarray(inputs[k], dtype=F32) for k in
         ("W_cls", "b_cls", "W_pos", "b_pos", "W_in", "b_in", "W_out", "b_out",
          "W_e1", "b_e1", "W_e2", "b_e2", "W_n1", "b_n1", "W_n2", "b_n2",
          "W_head", "b_head")}

    out = np.zeros((B, N), dtype=F32)

    perms, Kbs = [], []
    for b in range(B):
        perm = np.lexsort((-aid[b].astype(np.int64), -cls[b]))
        perms.append(perm)
        Kbs.append(int((cls[b] >= F32(CONF_THRES)).sum()))
    Kmax = max(Kbs)
    if Kmax == 0:
        return out

    K = min(N, 16 * ((Kmax + 15) // 16))
    n_t = K // 16
    J = K // 2
    Ls = _lens(n_t)
    groups = _groups(n_t)
    Lsum = sum(Ls)

    nc = _get_program(n_t)
    from concourse.bass_utils import run_bass_kernel_spmd

    be2 = float(w["b_e2"][0])
    we2d = np.zeros((I_DIM, TJ * TJ), dtype=F32)              # one-hot blocks
    for jj in range(TJ):
        we2d[:, TJ * jj + jj] = w["W_e2"][:, 0]
    we2_16 = we2d.astype(BF16)
    i8 = np.eye(TJ, dtype=BF16)

    in_maps = []
    for b in range(B):
        perm = perms[b]
        bf_s = bf[b][perm][:K]                 # [K, 64]
        e0_s = emb[b][perm, 0][:K]
        e1_s = emb[b][perm, 1][:K]
        ang_s = (e0_s * F32(np.pi)).astype(F32)
        pos_s = _pos_emb(e0_s, e1_s)           # [K, 2]

        feats = np.maximum(bf_s @ w["W_cls"] + w["b_cls"], 0.0).astype(F32)
        base = (pos_s @ w["W_pos"]).astype(F32)
        A = (feats @ w["W_in"] + base + (w["b_in"] + w["b_pos"])).astype(F32)
        C = (feats @ w["W_out"] + base + w["b_out"]).astype(F32)
        p = (A @ w["W_e1"]).astype(F32)                        # [K, 128]
        qneg_full = (w["b_e1"] - C @ w["W_e1"]).astype(F32)    # [K, 128]

        p16 = np.ascontiguousarray(p.T).astype(BF16)           # [128, K]

        iota = np.arange(K)
        for P in range(2):
            ranks = 2 * np.arange(J) + P                       # [J]
            qneg = np.ascontiguousarray(qneg_full[ranks].T)    # [128, J] f32
            # allowed[c, i] = |ang_i - ang_rank_c| < 0.5  and  i < rank_c
            dif = np.abs(ang_s[None, :] - ang_s[ranks][:, None]).astype(F32)
            allowed = (dif < F32(0.5)) & (iota[None, :] < ranks[:, None])
            madd = np.full((TJ, Lsum), MASK_NEG, dtype=F32)
            goff = 0
            for ts, wg in groups:
                off = 0
                for t in ts:
                    L = Ls[t]
                    rows = allowed[TJ * t: TJ * t + TJ, :L]
                    blk = np.where(rows, be2, MASK_NEG)
                    madd[:, goff + off: goff + off + L] = blk
                    off += L
                goff += wg
            in_maps.append({
                "p16": p16,
                "qneg": qneg,
                "madd": madd.astype(BF16),
                "we2": we2_16,
                "i8": i8,
            })

    res = run_bass_kernel_spmd(nc, in_maps, list(range(N_CORES)))

    for ci in range(N_CORES):
        b, P = ci // 2, ci % 2
        ymat = np.asarray(res.results[ci]["y"], dtype=F32)     # [TJ, n_t]
        nm = np.maximum(ymat.T.reshape(-1), 0.0).astype(F32)   # [J] c=8t+jj
        s1 = np.maximum(nm[:, None] * w["W_n1"][0][None, :] + w["b_n1"],
                        0.0).astype(F32)
        s2 = np.maximum(s1 @ w["W_n2"] + w["b_n2"], 0.0).astype(F32)
        logit = (s2 @ w["W_head"][:, 0] + w["b_head"][0]).astype(F32)
        prob = (1.0 / (1.0 + np.exp(-logit.astype(np.float64)))).astype(F32)
        ranks = 2 * np.arange(J) + P
        valid = ranks < Kbs[b]
        out[b, perms[b][ranks[valid]]] = prob[valid]
    return out


# revision 24
# speedup vs baseline: 3.0405x; 3.0405x over previous
"""Trainium2 Bass kernel for the O2O classification head (GNN message passing).

Strategy (v2)
-------------
The edge tensor is rank-structured: after the first edge-MLP layer the
pre-gelu value for pair (i, j) is u = p_i + qneg_j with per-node vectors
    p    = (feats@W_in + pos@W_pos + b_in + b_pos) @ W_e1
    qneg = b_e1 - (feats@W_out + pos@W_pos + b_out) @ W_e1
so the device only computes gelu(p_i + qneg_j) . W_e2 per pair.

Host-side, nodes are sorted by (cls desc, id desc); suppress[i,j] != 0
requires rank_i < rank_j.  Ranks with cls < 0.4 have output exactly
sigmoid(-1e6) = 0, and sorting puts them at ranks >= K, so only the
top-K block is computed at all (K = #{cls >= 0.4} rounded up to 16).

Sharding: 2 cores per batch; core parity P takes ranks == P (mod 2).
Per core, j's are processed in t-blocks of 8 with i-prefix 16(t+1).
All pair work is bf16: DVE broadcasts u = p + qneg (4x mode), Act does
gelu, PE dot-products with W_e2 accumulate into PSUM on top of a
host-precomputed additive mask (0 where allowed, -30000 elsewhere,
b_e2 folded in) injected via an identity-stationary matmul.  A row-max
per t-block then gives node_max; the tiny 64-wide output MLP and the
sigmoid run on host.
"""

import sys
import numpy as np
import ml_dtypes

if "/opt/trn_rl_repo" not in sys.path:
    sys.path.insert(0, "/opt/trn_rl_repo")

B, N = 4, 512
H_DIM, I_DIM = 64, 128
N_CORES = 8
TJ = 8               # j's per t-block
GROUP_W = 512        # max PSUM bank width (fp32 cols)
F32 = np.float32
BF16 = ml_dtypes.bfloat16

IMG_W, IMG_H, CENTER_H = 800.0, 320.0, 160.0
NUM_OFFSETS = 72
CONF_THRES = 0.4
MASK_NEG = -30000.0

_PROGRAMS = {}       # n_t -> compiled program
_LAST_NT = None


def _lens(n_t):
    return [16 * (t + 1) for t in range(n_t)]


def _groups(n_t):
    """Greedy grouping of consecutive t-blocks with sum(L) <= GROUP_W."""
    Ls = _lens(n_t)
    groups, cur, cur_w = [], [], 0
    for t in range(n_t):
        if cur and cur_w + Ls[t] > GROUP_W:
            groups.append((cur, cur_w))
            cur, cur_w = [], 0
        cur.append(t)
        cur_w += Ls[t]
    groups.append((cur, cur_w))
    return groups


def _build_program(n_t, num_devices=N_CORES):
    import contextlib
    import concourse.bass as bass  # noqa: F401
    import concourse.tile as tile
    from concourse import bacc, mybir

    f32 = mybir.dt.float32
    bf16 = mybir.dt.bfloat16
    AF = mybir.ActivationFunctionType
    AX = mybir.AxisListType

    Ls = _lens(n_t)
    groups = _groups(n_t)
    Lsum = sum(Ls)
    J = TJ * n_t
    K = 2 * J

    nc = bacc.Bacc("TRN2", target_bir_lowering=False, debug=False,
                   num_devices=num_devices)

    d_p16 = nc.declare_dram_parameter("p16", [I_DIM, K], bf16, isOutput=False)
    d_qneg = nc.declare_dram_parameter("qneg", [I_DIM, J], f32, isOutput=False)
    d_madd = nc.declare_dram_parameter("madd", [TJ, Lsum], bf16,
                                       isOutput=False)
    d_we2 = nc.declare_dram_parameter("we2", [I_DIM, TJ * TJ], bf16,
                                      isOutput=False)
    d_i8 = nc.declare_dram_parameter("i8", [TJ, TJ], bf16, isOutput=False)
    y = nc.declare_dram_parameter("y", [TJ, n_t], f32, isOutput=True)

    with tile.TileContext(nc) as tc:
        with contextlib.ExitStack() as ctx:
            const = ctx.enter_context(tc.tile_pool(name="const", bufs=1))
            ub = ctx.enter_context(tc.tile_pool(name="ub", bufs=3))
            gb = ctx.enter_context(tc.tile_pool(name="gb", bufs=3))
            sp = ctx.enter_context(tc.tile_pool(name="sp", bufs=4,
                                                space="PSUM"))

            p16 = const.tile([I_DIM, K], bf16, name="p16", tag="p16")
            qneg = const.tile([I_DIM, J], f32, name="qneg", tag="qneg")
            madd = const.tile([TJ, Lsum], bf16, name="madd", tag="madd")
            we2 = const.tile([I_DIM, TJ * TJ], bf16, name="we2", tag="we2")
            i8 = const.tile([TJ, TJ], bf16, name="i8", tag="i8")
            # spread input DMAs across engine queues so they load in parallel
            nc.gpsimd.dma_start(out=p16[:], in_=d_p16[:])
            nc.sync.dma_start(out=qneg[:], in_=d_qneg[:])
            nc.sync.dma_start(out=madd[:], in_=d_madd[:])
            nc.sync.dma_start(out=we2[:], in_=d_we2[:])
            nc.sync.dma_start(out=i8[:], in_=d_i8[:])

            nmall = const.tile([TJ, n_t], f32, name="nmall", tag="nmall")

            UW = TJ * GROUP_W
            pending = []

            def flush_one():
                S, ts, offs, w = pending.pop(0)
                for t, off in zip(ts, offs):
                    nc.vector.reduce_max(nmall[:, t:t + 1],
                                         S[:, off:off + Ls[t]], axis=AX.X)

            goff = 0
            for gi, (ts, w) in enumerate(groups):
                U = ub.tile([I_DIM, UW], bf16, name=f"U{gi}", tag="U")
                G = gb.tile([I_DIM, UW], bf16, name=f"G{gi}", tag="G")
                off = 0
                offs = []
                for t in ts:
                    L = Ls[t]
                    for jj in range(TJ):
                        nc.vector.tensor_scalar_add(
                            U[:, jj * w + off: jj * w + off + L],
                            p16[:, :L],
                            qneg[:, TJ * t + jj: TJ * t + jj + 1])
                    offs.append(off)
                    off += L
                nc.scalar.activation(G[:, :TJ * w], U[:, :TJ * w], AF.Gelu)

                S = sp.tile([TJ, GROUP_W], f32, name=f"S{gi}", tag="S")
                nc.tensor.matmul(S[:, :w], i8[:], madd[:, goff:goff + w],
                                 start=True, stop=False)
                for jj in range(TJ):
                    nc.tensor.matmul(S[:, :w], we2[:, TJ * jj: TJ * (jj + 1)],
                                     G[:, jj * w: (jj + 1) * w],
                                     start=False, stop=(jj == TJ - 1))
                pending.append((S, ts, offs, w))
                goff += w
                if len(pending) > 2:
                    flush_one()

            while pending:
                flush_one()

            nc.gpsimd.dma_start(out=y[:], in_=nmall[:])

    nc.compile()
    return nc


def _get_program(n_t=None):
    global _LAST_NT
    if n_t is None:
        n_t = _LAST_NT
    if n_t not in _PROGRAMS:
        _PROGRAMS[n_t] = _build_program(n_t)
    _LAST_NT = n_t
    return _PROGRAMS[n_t]


def _pos_emb(e0, e1):
    """float32 mirror of the reference _get_sample_point (one batch, sorted)."""
    angle = (e0 * F32(np.pi)).astype(F32)
    rho = (e1 * F32(IMG_W)).astype(F32)
    lin = np.linspace(0.0, 1.0 - 1e-5, NUM_OFFSETS, dtype=F32)
    yk = (F32(CENTER_H) - lin * F32(IMG_H)).astype(F32)[:2]
    tan = np.tan(angle, dtype=F32)
    roc = (rho / np.cos(angle, dtype=F32)).astype(F32)
    x = (-tan[:, None] * yk[None, :] + roc[:, None]).astype(F32)
    return (x / F32(IMG_W)).astype(F32)          # [n, 2]


def kernel(**inputs):
    global _LAST_NT
    bf = np.asarray(inputs["batch_features"], dtype=F32)      # [B,N,64]
    cls = np.asarray(inputs["cls_pred"], dtype=F32)           # [B,N]
    aid = np.asarray(inputs["anchor_id"])                     # [B,N] int32
    emb = np.asarray(inputs["anchor_embeddings"], dtype=F32)  # [B,N,2]

    w = {k: np.asarray(inputs[k], dtype=F32) for k in
         ("W_cls", "b_cls", "W_pos", "b_pos", "W_in", "b_in", "W_out", "b_out",
          "W_e1", "b_e1", "W_e2", "b_e2", "W_n1", "b_n1", "W_n2", "b_n2",
          "W_head", "b_head")}

    out = np.zeros((B, N), dtype=F32)

    perms, Kbs = [], []
    for b in range(B):
        perm = np.lexsort((-aid[b].astype(np.int64), -cls[b]))
        perms.append(perm)
        Kbs.append(int((cls[b] >= F32(CONF_THRES)).sum()))
    Kmax = max(Kbs)
    if Kmax == 0:
        return out

    K = min(N, 16 * ((Kmax + 15) // 16))
    n_t = K // 16
    J = K // 2
    Ls = _lens(n_t)
    groups = _groups(n_t)
    Lsum = sum(Ls)

    nc = _get_program(n_t)
    from concourse.bass_utils import run_bass_kernel_spmd

    be2 = float(w["b_e2"][0])
    we2d = np.zeros((I_DIM, TJ * TJ), dtype=F32)              # one-hot blocks
    for jj in range(TJ):
        we2d[:, TJ * jj + jj] = w["W_e2"][:, 0]
    we2_16 = we2d.astype(BF16)
    i8 = np.eye(TJ, dtype=BF16)

    in_maps = []
    for b in range(B):
        perm = perms[b]
        bf_s = bf[b][perm][:K]                 # [K, 64]
        e0_s = emb[b][perm, 0][:K]
        e1_s = emb[b][perm, 1][:K]
        ang_s = (e0_s * F32(np.pi)).astype(F32)
        pos_s = _pos_emb(e0_s, e1_s)           # [K, 2]

        feats = np.maximum(bf_s @ w["W_cls"] + w["b_cls"], 0.0).astype(F32)
        base = (pos_s @ w["W_pos"]).astype(F32)
        A = (feats @ w["W_in"] + base + (w["b_in"] + w["b_pos"])).astype(F32)
        C = (feats @ w["W_out"] + base + w["b_out"]).astype(F32)
        p = (A @ w["W_e1"]).astype(F32)                        # [K, 128]
        qneg_full = (w["b_e1"] - C @ w["W_e1"]).astype(F32)    # [K, 128]

        p16 = np.ascontiguousarray(p.T).astype(BF16)           # [128, K]

        iota = np.arange(K)
        for P in range(2):
            ranks = 2 * np.arange(J) + P                       # [J]
            qneg = np.ascontiguousarray(qneg_full[ranks].T)    # [128, J] f32
            # allowed[c, i] = |ang_i - ang_rank_c| < 0.5  and  i < rank_c
            dif = np.abs(ang_s[None, :] - ang_s[ranks][:, None]).astype(F32)
            allowed = (dif < F32(0.5)) & (iota[None, :] < ranks[:, None])
            madd = np.full((TJ, Lsum), MASK_NEG, dtype=F32)
            goff = 0
            for ts, wg in groups:
                off = 0
                for t in ts:
                    L = Ls[t]
                    rows = allowed[TJ * t: TJ * t + TJ, :L]
                    blk = np.where(rows, be2, MASK_NEG)
                    madd[:, goff + off: goff + off + L] = blk
                    off += L
                goff += wg
            in_maps.append({
                "p16": p16,
                "qneg": qneg,
                "madd": madd.astype(BF16),
                "we2": we2_16,
                "i8": i8,
            })

    res = run_bass_kernel_spmd(nc, in_maps, list(range(N_CORES)))

    for ci in range(N_CORES):
        b, P = ci // 2, ci % 2
        ymat = np.asarray(res.results[ci]["y"], dtype=F32)     # [TJ, n_t]
        nm = np.maximum(ymat.T.reshape(-1), 0.0).astype(F32)   # [J] c=8t+jj
        s1 = np.maximum(nm[:, None] * w["W_n1"][0][None, :] + w["b_n1"],
                        0.0).astype(F32)
        s2 = np.maximum(s1 @ w["W_n2"] + w["b_n2"], 0.0).astype(F32)
        logit = (s2 @ w["W_head"][:, 0] + w["b_head"][0]).astype(F32)
        prob = (1.0 / (1.0 + np.exp(-logit.astype(np.float64)))).astype(F32)
        ranks = 2 * np.arange(J) + P
        valid = ranks < Kbs[b]
        out[b, perms[b][ranks[valid]]] = prob[valid]
    return out


# revision 28
# speedup vs baseline: 3.4220x; 1.1255x over previous
"""Trainium2 Bass kernel for the O2O classification head (GNN message passing).

Strategy (v2)
-------------
The edge tensor is rank-structured: after the first edge-MLP layer the
pre-gelu value for pair (i, j) is u = p_i + qneg_j with per-node vectors
    p    = (feats@W_in + pos@W_pos + b_in + b_pos) @ W_e1
    qneg = b_e1 - (feats@W_out + pos@W_pos + b_out) @ W_e1
so the device only computes gelu(p_i + qneg_j) . W_e2 per pair.

Host-side, nodes are sorted by (cls desc, id desc); suppress[i,j] != 0
requires rank_i < rank_j.  Ranks with cls < 0.4 have output exactly
sigmoid(-1e6) = 0, and sorting puts them at ranks >= K, so only the
top-K block is computed at all (K = #{cls >= 0.4} rounded up to 16).

Sharding: 2 cores per batch; core parity P takes ranks == P (mod 2).
Per core, j's are processed in t-blocks of 8 with i-prefix 16(t+1).
All pair work is bf16: DVE broadcasts u = p + qneg (4x mode), Act does
gelu, PE dot-products with W_e2 accumulate into PSUM on top of a
host-precomputed additive mask (0 where allowed, -30000 elsewhere,
b_e2 folded in) injected via an identity-stationary matmul.  A row-max
per t-block then gives node_max; the tiny 64-wide output MLP and the
sigmoid run on host.
"""

import sys
import numpy as np
import ml_dtypes

if "/opt/trn_rl_repo" not in sys.path:
    sys.path.insert(0, "/opt/trn_rl_repo")

B, N = 4, 512
H_DIM, I_DIM = 64, 128
N_CORES = 8
TJ = 8               # j's per t-block
GROUP_W = 512        # max PSUM bank width (fp32 cols)
F32 = np.float32
BF16 = ml_dtypes.bfloat16

IMG_W, IMG_H, CENTER_H = 800.0, 320.0, 160.0
NUM_OFFSETS = 72
CONF_THRES = 0.4
MASK_NEG = -30000.0

_PROGRAMS = {}       # n_t -> compiled program
_LAST_NT = None


def _lens(n_t):
    return [16 * (t + 1) for t in range(n_t)]


def _groups(n_t):
    """Greedy grouping of consecutive t-blocks with sum(L) <= GROUP_W.

    Processing order: second-cheapest group first (activation starts early),
    cheapest last (short pipeline tail), the rest in between.
    """
    Ls = _lens(n_t)
    groups, cur, cur_w = [], [], 0
    for t in range(n_t):
        if cur and cur_w + Ls[t] > GROUP_W:
            groups.append((cur, cur_w))
            cur, cur_w = [], 0
        cur.append(t)
        cur_w += Ls[t]
    groups.append((cur, cur_w))
    if len(groups) >= 3:
        order = sorted(range(len(groups)), key=lambda g: groups[g][1])
        first, last = order[1], order[0]
        mid = [g for g in range(len(groups)) if g not in (first, last)]
        groups = [groups[first]] + [groups[g] for g in mid] + [groups[last]]
    return groups


def _build_program(n_t, num_devices=N_CORES):
    import contextlib
    import concourse.bass as bass  # noqa: F401
    import concourse.tile as tile
    from concourse import bacc, mybir

    f32 = mybir.dt.float32
    bf16 = mybir.dt.bfloat16
    AF = mybir.ActivationFunctionType
    AX = mybir.AxisListType

    Ls = _lens(n_t)
    groups = _groups(n_t)
    Lsum = sum(Ls)
    J = TJ * n_t
    K = 2 * J

    nc = bacc.Bacc("TRN2", target_bir_lowering=False, debug=False,
                   num_devices=num_devices)

    d_p16 = nc.declare_dram_parameter("p16", [I_DIM, K], bf16, isOutput=False)
    d_qneg = nc.declare_dram_parameter("qneg", [I_DIM, J], f32, isOutput=False)
    d_madd = nc.declare_dram_parameter("madd", [TJ, Lsum], bf16,
                                       isOutput=False)
    d_we2 = nc.declare_dram_parameter("we2", [I_DIM, TJ * TJ], bf16,
                                      isOutput=False)
    d_i8 = nc.declare_dram_parameter("i8", [TJ, TJ], bf16, isOutput=False)
    y = nc.declare_dram_parameter("y", [TJ, n_t], f32, isOutput=True)

    with tile.TileContext(nc) as tc:
        with contextlib.ExitStack() as ctx:
            const = ctx.enter_context(tc.tile_pool(name="const", bufs=1))
            ub = ctx.enter_context(tc.tile_pool(name="ub", bufs=4))
            gb = ctx.enter_context(tc.tile_pool(name="gb", bufs=4))
            sp = ctx.enter_context(tc.tile_pool(name="sp", bufs=6,
                                                space="PSUM"))

            p16 = const.tile([I_DIM, K], bf16, name="p16", tag="p16")
            qneg = const.tile([I_DIM, J], f32, name="qneg", tag="qneg")
            madd = const.tile([TJ, Lsum], bf16, name="madd", tag="madd")
            we2 = const.tile([I_DIM, TJ * TJ], bf16, name="we2", tag="we2")
            i8 = const.tile([TJ, TJ], bf16, name="i8", tag="i8")
            # spread input DMAs across engine queues so they load in parallel
            nc.gpsimd.dma_start(out=p16[:], in_=d_p16[:])
            nc.sync.dma_start(out=qneg[:], in_=d_qneg[:])
            nc.sync.dma_start(out=madd[:], in_=d_madd[:])
            nc.sync.dma_start(out=we2[:], in_=d_we2[:])
            nc.sync.dma_start(out=i8[:], in_=d_i8[:])

            nmall = const.tile([TJ, n_t], f32, name="nmall", tag="nmall")

            UW = TJ * GROUP_W
            pending = []

            def flush_one():
                S, ts, offs, w = pending.pop(0)
                for t, off in zip(ts, offs):
                    nc.vector.reduce_max(nmall[:, t:t + 1],
                                         S[:, off:off + Ls[t]], axis=AX.X)

            goff = 0
            for gi, (ts, w) in enumerate(groups):
                U = ub.tile([I_DIM, UW], bf16, name=f"U{gi}", tag="U")
                G = gb.tile([I_DIM, UW], bf16, name=f"G{gi}", tag="G")
                off = 0
                offs = []
                for t in ts:
                    L = Ls[t]
                    for jj in range(TJ):
                        nc.vector.tensor_scalar_add(
                            U[:, jj * w + off: jj * w + off + L],
                            p16[:, :L],
                            qneg[:, TJ * t + jj: TJ * t + jj + 1])
                    offs.append(off)
                    off += L
                half = (TJ // 2) * w
                nc.scalar.activation(G[:, :half], U[:, :half], AF.Gelu)
                nc.scalar.activation(G[:, half:TJ * w], U[:, half:TJ * w],
                                     AF.Gelu)

                S = sp.tile([TJ, GROUP_W], f32, name=f"S{gi}", tag="S")
                nc.tensor.matmul(S[:, :w], i8[:], madd[:, goff:goff + w],
                                 start=True, stop=False)
                for jj in range(TJ):
                    nc.tensor.matmul(S[:, :w], we2[:, TJ * jj: TJ * (jj + 1)],
                                     G[:, jj * w: (jj + 1) * w],
                                     start=False, stop=(jj == TJ - 1))
                pending.append((S, ts, offs, w))
                goff += w
                if len(pending) > 4:
                    flush_one()

            while pending:
                flush_one()

            nc.gpsimd.dma_start(out=y[:], in_=nmall[:])

    nc.compile()
    return nc


def _get_program(n_t=None):
    global _LAST_NT
    if n_t is None:
        n_t = _LAST_NT
    if n_t not in _PROGRAMS:
        _PROGRAMS[n_t] = _build_program(n_t)
    _LAST_NT = n_t
    return _PROGRAMS[n_t]


def _pos_emb(e0, e1):
    """float32 mirror of the reference _get_sample_point (one batch, sorted)."""
    angle = (e0 * F32(np.pi)).astype(F32)
    rho = (e1 * F32(IMG_W)).astype(F32)
    lin = np.linspace(0.0, 1.0 - 1e-5, NUM_OFFSETS, dtype=F32)
    yk = (F32(CENTER_H) - lin * F32(IMG_H)).astype(F32)[:2]
    tan = np.tan(angle, dtype=F32)
    roc = (rho / np.cos(angle, dtype=F32)).astype(F32)
    x = (-tan[:, None] * yk[None, :] + roc[:, None]).astype(F32)
    return (x / F32(IMG_W)).astype(F32)          # [n, 2]


def kernel(**inputs):
    global _LAST_NT
    bf = np.asarray(inputs["batch_features"], dtype=F32)      # [B,N,64]
    cls = np.asarray(inputs["cls_pred"], dtype=F32)           # [B,N]
    aid = np.asarray(inputs["anchor_id"])                     # [B,N] int32
    emb = np.asarray(inputs["anchor_embeddings"], dtype=F32)  # [B,N,2]

    w = {k: np.asarray(inputs[k], dtype=F32) for k in
         ("W_cls", "b_cls", "W_pos", "b_pos", "W_in", "b_in", "W_out", "b_out",
          "W_e1", "b_e1", "W_e2", "b_e2", "W_n1", "b_n1", "W_n2", "b_n2",
          "W_head", "b_head")}

    out = np.zeros((B, N), dtype=F32)

    perms, Kbs = [], []
    for b in range(B):
        perm = np.lexsort((-aid[b].astype(np.int64), -cls[b]))
        perms.append(perm)
        Kbs.append(int((cls[b] >= F32(CONF_THRES)).sum()))
    Kmax = max(Kbs)
    if Kmax == 0:
        return out

    K = min(N, 16 * ((Kmax + 15) // 16))
    n_t = K // 16
    J = K // 2
    Ls = _lens(n_t)
    groups = _groups(n_t)
    Lsum = sum(Ls)

    nc = _get_program(n_t)
    from concourse.bass_utils import run_bass_kernel_spmd

    be2 = float(w["b_e2"][0])
    we2d = np.zeros((I_DIM, TJ * TJ), dtype=F32)              # one-hot blocks
    for jj in range(TJ):
        we2d[:, TJ * jj + jj] = w["W_e2"][:, 0]
    we2_16 = we2d.astype(BF16)
    i8 = np.eye(TJ, dtype=BF16)

    in_maps = []
    for b in range(B):
        perm = perms[b]
        bf_s = bf[b][perm][:K]                 # [K, 64]
        e0_s = emb[b][perm, 0][:K]
        e1_s = emb[b][perm, 1][:K]
        ang_s = (e0_s * F32(np.pi)).astype(F32)
        pos_s = _pos_emb(e0_s, e1_s)           # [K, 2]

        feats = np.maximum(bf_s @ w["W_cls"] + w["b_cls"], 0.0).astype(F32)
        base = (pos_s @ w["W_pos"]).astype(F32)
        A = (feats @ w["W_in"] + base + (w["b_in"] + w["b_pos"])).astype(F32)
        C = (feats @ w["W_out"] + base + w["b_out"]).astype(F32)
        p = (A @ w["W_e1"]).astype(F32)                        # [K, 128]
        qneg_full = (w["b_e1"] - C @ w["W_e1"]).astype(F32)    # [K, 128]

        p16 = np.ascontiguousarray(p.T).astype(BF16)           # [128, K]

        iota = np.arange(K)
        for P in range(2):
            ranks = 2 * np.arange(J) + P                       # [J]
            qneg = np.ascontiguousarray(qneg_full[ranks].T)    # [128, J] f32
            # allowed[c, i] = |ang_i - ang_rank_c| < 0.5  and  i < rank_c
            dif = np.abs(ang_s[None, :] - ang_s[ranks][:, None]).astype(F32)
            allowed = (dif < F32(0.5)) & (iota[None, :] < ranks[:, None])
            madd = np.full((TJ, Lsum), MASK_NEG, dtype=F32)
            goff = 0
            for ts, wg in groups:
                off = 0
                for t in ts:
                    L = Ls[t]
                    rows = allowed[TJ * t: TJ * t + TJ, :L]
                    blk = np.where(rows, be2, MASK_NEG)
                    madd[:, goff + off: goff + off + L] = blk
                    off += L
                goff += wg
            in_maps.append({
                "p16": p16,
                "qneg": qneg,
                "madd": madd.astype(BF16),
                "we2": we2_16,
                "i8": i8,
            })

    res = run_bass_kernel_spmd(nc, in_maps, list(range(N_CORES)))

    for ci in range(N_CORES):
        b, P = ci // 2, ci % 2
        ymat = np.asarray(res.results[ci]["y"], dtype=F32)     # [TJ, n_t]
        nm = np.maximum(ymat.T.reshape(-1), 0.0).astype(F32)   # [J] c=8t+jj
        s1 = np.maximum(nm[:, None] * w["W_n1"][0][None, :] + w["b_n1"],
                        0.0).astype(F32)
        s2 = np.maximum(s1 @ w["W_n2"] + w["b_n2"], 0.0).astype(F32)
        logit = (s2 @ w["W_head"][:, 0] + w["b_head"][0]).astype(F32)
        prob = (1.0 / (1.0 + np.exp(-logit.astype(np.float64)))).astype(F32)
        ranks = 2 * np.arange(J) + P
        valid = ranks < Kbs[b]
        out[b, perms[b][ranks[valid]]] = prob[valid]
    return out


# revision 42
# speedup vs baseline: 3.5160x; 1.0275x over previous
"""Trainium2 Bass kernel for the O2O classification head (GNN message passing).

Strategy (v2)
-------------
The edge tensor is rank-structured: after the first edge-MLP layer the
pre-gelu value for pair (i, j) is u = p_i + qneg_j with per-node vectors
    p    = (feats@W_in + pos@W_pos + b_in + b_pos) @ W_e1
    qneg = b_e1 - (feats@W_out + pos@W_pos + b_out) @ W_e1
so the device only computes gelu(p_i + qneg_j) . W_e2 per pair.

Host-side, nodes are sorted by (cls desc, id desc); suppress[i,j] != 0
requires rank_i < rank_j.  Ranks with cls < 0.4 have output exactly
sigmoid(-1e6) = 0, and sorting puts them at ranks >= K, so only the
top-K block is computed at all (K = #{cls >= 0.4} rounded up to 16).

Sharding: 2 cores per batch; core parity P takes ranks == P (mod 2).
Per core, j's are processed in t-blocks of 8 with i-prefix 16(t+1).
All pair work is bf16: DVE broadcasts u = p + qneg (4x mode), Act does
gelu, PE dot-products with W_e2 accumulate into PSUM on top of a
host-precomputed additive mask (0 where allowed, -30000 elsewhere,
b_e2 folded in) injected via an identity-stationary matmul.  A row-max
per t-block then gives node_max; the tiny 64-wide output MLP and the
sigmoid run on host.
"""

import sys
import numpy as np
import ml_dtypes

if "/opt/trn_rl_repo" not in sys.path:
    sys.path.insert(0, "/opt/trn_rl_repo")

B, N = 4, 512
H_DIM, I_DIM = 64, 128
N_CORES = 8
TJ = 8               # j's per t-block
GROUP_W = 512        # max PSUM bank width (fp32 cols)
F32 = np.float32
BF16 = ml_dtypes.bfloat16

IMG_W, IMG_H, CENTER_H = 800.0, 320.0, 160.0
NUM_OFFSETS = 72
CONF_THRES = 0.4
MASK_NEG = -30000.0

_PROGRAMS = {}       # n_t -> compiled program
_LAST_NT = None


def _lens(n_t):
    return [16 * (t + 1) for t in range(n_t)]


def _psum_ts(n_t):
    """t-blocks whose gelu input is built on the PE in PSUM (not DVE)."""
    return set(range(3)) if n_t >= 6 else set()


def _groups(n_t):
    """Group t-blocks into PSUM-bank-sized column groups.

    PE-offloaded t-blocks become singleton groups processed first; the rest
    are greedily packed (sum L <= GROUP_W) with the second-cheapest group
    first (activation starts early) and the cheapest last (short tail).
    """
    Ls = _lens(n_t)
    pts = _psum_ts(n_t)
    groups, cur, cur_w = [], [], 0
    for t in range(n_t):
        if t in pts:
            continue
        if cur and cur_w + Ls[t] > GROUP_W:
            groups.append((cur, cur_w))
            cur, cur_w = [], 0
        cur.append(t)
        cur_w += Ls[t]
    groups.append((cur, cur_w))
    if len(groups) >= 3:
        order = sorted(range(len(groups)), key=lambda g: groups[g][1])
        first, last = order[1], order[0]
        mid = [g for g in range(len(groups)) if g not in (first, last)]
        groups = [groups[first]] + [groups[g] for g in mid] + [groups[last]]
    return [([t], Ls[t]) for t in sorted(pts)] + groups


def _build_program(n_t, num_devices=N_CORES):
    import contextlib
    import concourse.bass as bass  # noqa: F401
    import concourse.tile as tile
    from concourse import bacc, mybir

    f32 = mybir.dt.float32
    bf16 = mybir.dt.bfloat16
    AF = mybir.ActivationFunctionType
    AX = mybir.AxisListType

    Ls = _lens(n_t)
    groups = _groups(n_t)
    Lsum = sum(Ls)
    J = TJ * n_t
    K = 2 * J

    nc = bacc.Bacc("TRN2", target_bir_lowering=False, debug=False,
                   num_devices=num_devices)

    d_p16 = nc.declare_dram_parameter("p16", [I_DIM, K], bf16, isOutput=False)
    d_qneg = nc.declare_dram_parameter("qneg", [I_DIM, J], f32, isOutput=False)
    d_madd = nc.declare_dram_parameter("madd", [TJ, Lsum], bf16,
                                       isOutput=False)
    d_we2 = nc.declare_dram_parameter("we2", [I_DIM, TJ * TJ], bf16,
                                      isOutput=False)
    d_i8 = nc.declare_dram_parameter("i8", [TJ, TJ], bf16, isOutput=False)
    n_qnt = max(TJ * len(_psum_ts(n_t)), 1)
    d_qnt = nc.declare_dram_parameter("qnegT", [1, n_qnt * I_DIM], bf16,
                                      isOutput=False)
    d_i128 = nc.declare_dram_parameter("i128", [I_DIM, I_DIM], bf16,
                                       isOutput=False)
    d_ones = nc.declare_dram_parameter("ones1", [1, 64], bf16, isOutput=False)
    y = nc.declare_dram_parameter("y", [TJ, n_t], f32, isOutput=True)

    with tile.TileContext(nc) as tc:
        with contextlib.ExitStack() as ctx:
            const = ctx.enter_context(tc.tile_pool(name="const", bufs=1))
            ub = ctx.enter_context(tc.tile_pool(name="ub", bufs=4))
            gb = ctx.enter_context(tc.tile_pool(name="gb", bufs=4))
            sp = ctx.enter_context(tc.tile_pool(name="sp", bufs=6,
                                                space="PSUM"))
            up = ctx.enter_context(tc.tile_pool(name="up", bufs=2,
                                                space="PSUM"))

            p16 = const.tile([I_DIM, K], bf16, name="p16", tag="p16")
            qneg = const.tile([I_DIM, J], f32, name="qneg", tag="qneg")
            madd = const.tile([TJ, Lsum], bf16, name="madd", tag="madd")
            we2 = const.tile([I_DIM, TJ * TJ], bf16, name="we2", tag="we2")
            i8 = const.tile([TJ, TJ], bf16, name="i8", tag="i8")
            qnt = const.tile([1, n_qnt * I_DIM], bf16, name="qnt", tag="qnt")
            i128 = const.tile([I_DIM, I_DIM], bf16, name="i128", tag="i128")
            ones1 = const.tile([1, 64], bf16, name="ones1", tag="ones1")
            # spread input DMAs across engine queues so they load in parallel
            nc.gpsimd.dma_start(out=p16[:], in_=d_p16[:])
            nc.gpsimd.dma_start(out=i128[:], in_=d_i128[:])
            nc.gpsimd.dma_start(out=ones1[:], in_=d_ones[:])
            nc.sync.dma_start(out=qneg[:], in_=d_qneg[:])
            nc.sync.dma_start(out=qnt[:], in_=d_qnt[:])
            nc.sync.dma_start(out=madd[:], in_=d_madd[:])
            nc.sync.dma_start(out=we2[:], in_=d_we2[:])
            nc.sync.dma_start(out=i8[:], in_=d_i8[:])

            nmall = const.tile([TJ, n_t], f32, name="nmall", tag="nmall")

            UW = TJ * GROUP_W
            pending = []

            def flush_one():
                S, ts, offs, w = pending.pop(0)
                for t, off in zip(ts, offs):
                    nc.vector.reduce_max(nmall[:, t:t + 1],
                                         S[:, off:off + Ls[t]], axis=AX.X)

            pts = _psum_ts(n_t)
            goff = 0
            for gi, (ts, w) in enumerate(groups):
                G = gb.tile([I_DIM, UW], bf16, name=f"G{gi}", tag="G")
                if len(ts) == 1 and ts[0] in pts:
                    # PE builds gelu input in PSUM: copy p16 (identity
                    # stationary) then broadcast qneg (rank-1 matmul)
                    t = ts[0]
                    L = Ls[t]
                    UP = up.tile([I_DIM, TJ * L], f32, name=f"UP{gi}",
                                 tag="UP")
                    for jj in range(TJ):
                        nc.tensor.matmul(UP[:, jj * L:(jj + 1) * L], i128[:],
                                         p16[:, :L], start=True, stop=False)
                    for jj in range(TJ):
                        c = TJ * t + jj
                        nc.tensor.matmul(UP[:, jj * L:(jj + 1) * L],
                                         qnt[:, c * I_DIM:(c + 1) * I_DIM],
                                         ones1[:, :L],
                                         start=False, stop=True)
                    nc.scalar.activation(G[:, :TJ * w], UP[:], AF.Gelu)
                    offs = [0]
                else:
                    U = ub.tile([I_DIM, UW], bf16, name=f"U{gi}", tag="U")
                    off = 0
                    offs = []
                    for t in ts:
                        L = Ls[t]
                        for jj in range(TJ):
                            nc.vector.tensor_scalar_add(
                                U[:, jj * w + off: jj * w + off + L],
                                p16[:, :L],
                                qneg[:, TJ * t + jj: TJ * t + jj + 1])
                        offs.append(off)
                        off += L
                    nc.scalar.activation(G[:, :TJ * w], U[:, :TJ * w],
                                         AF.Gelu)

                S = sp.tile([TJ, GROUP_W], f32, name=f"S{gi}", tag="S")
                nc.tensor.matmul(S[:, :w], i8[:], madd[:, goff:goff + w],
                                 start=True, stop=False)
                for jj in range(TJ):
                    nc.tensor.matmul(S[:, :w], we2[:, TJ * jj: TJ * (jj + 1)],
                                     G[:, jj * w: (jj + 1) * w],
                                     start=False, stop=(jj == TJ - 1))
                pending.append((S, ts, offs, w))
                goff += w
                if len(pending) > 4:
                    flush_one()

            while pending:
                flush_one()

            nc.gpsimd.dma_start(out=y[:], in_=nmall[:])

    nc.compile()
    return nc


def _get_program(n_t=None):
    global _LAST_NT
    if n_t is None:
        n_t = _LAST_NT
    if n_t not in _PROGRAMS:
        _PROGRAMS[n_t] = _build_program(n_t)
    _LAST_NT = n_t
    return _PROGRAMS[n_t]


def _pos_emb(e0, e1):
    """float32 mirror of the reference _get_sample_point (one batch, sorted)."""
    angle = (e0 * F32(np.pi)).astype(F32)
    rho = (e1 * F32(IMG_W)).astype(F32)
    lin = np.linspace(0.0, 1.0 - 1e-5, NUM_OFFSETS, dtype=F32)
    yk = (F32(CENTER_H) - lin * F32(IMG_H)).astype(F32)[:2]
    tan = np.tan(angle, dtype=F32)
    roc = (rho / np.cos(angle, dtype=F32)).astype(F32)
    x = (-tan[:, None] * yk[None, :] + roc[:, None]).astype(F32)
    return (x / F32(IMG_W)).astype(F32)          # [n, 2]


def kernel(**inputs):
    global _LAST_NT
    bf = np.asarray(inputs["batch_features"], dtype=F32)      # [B,N,64]
    cls = np.asarray(inputs["cls_pred"], dtype=F32)           # [B,N]
    aid = np.asarray(inputs["anchor_id"])                     # [B,N] int32
    emb = np.asarray(inputs["anchor_embeddings"], dtype=F32)  # [B,N,2]

    w = {k: np.asarray(inputs[k], dtype=F32) for k in
         ("W_cls", "b_cls", "W_pos", "b_pos", "W_in", "b_in", "W_out", "b_out",
          "W_e1", "b_e1", "W_e2", "b_e2", "W_n1", "b_n1", "W_n2", "b_n2",
          "W_head", "b_head")}

    out = np.zeros((B, N), dtype=F32)

    perms, Kbs = [], []
    for b in range(B):
        perm = np.lexsort((-aid[b].astype(np.int64), -cls[b]))
        perms.append(perm)
        Kbs.append(int((cls[b] >= F32(CONF_THRES)).sum()))
    Kmax = max(Kbs)
    if Kmax == 0:
        return out

    K = min(N, 16 * ((Kmax + 15) // 16))
    n_t = K // 16
    J = K // 2
    Ls = _lens(n_t)
    groups = _groups(n_t)
    Lsum = sum(Ls)

    nc = _get_program(n_t)
    from concourse.bass_utils import run_bass_kernel_spmd

    be2 = float(w["b_e2"][0])
    we2d = np.zeros((I_DIM, TJ * TJ), dtype=F32)              # one-hot blocks
    for jj in range(TJ):
        we2d[:, TJ * jj + jj] = w["W_e2"][:, 0]
    we2_16 = we2d.astype(BF16)
    i8 = np.eye(TJ, dtype=BF16)
    i128 = np.eye(I_DIM, dtype=BF16)
    ones1 = np.ones((1, 64), dtype=BF16)

    in_maps = []
    for b in range(B):
        perm = perms[b]
        bf_s = bf[b][perm][:K]                 # [K, 64]
        e0_s = emb[b][perm, 0][:K]
        e1_s = emb[b][perm, 1][:K]
        ang_s = (e0_s * F32(np.pi)).astype(F32)
        pos_s = _pos_emb(e0_s, e1_s)           # [K, 2]

        feats = np.maximum(bf_s @ w["W_cls"] + w["b_cls"], 0.0).astype(F32)
        base = (pos_s @ w["W_pos"]).astype(F32)
        A = (feats @ w["W_in"] + base + (w["b_in"] + w["b_pos"])).astype(F32)
        C = (feats @ w["W_out"] + base + w["b_out"]).astype(F32)
        p = (A @ w["W_e1"]).astype(F32)                        # [K, 128]
        qneg_full = (w["b_e1"] - C @ w["W_e1"]).astype(F32)    # [K, 128]

        p16 = np.ascontiguousarray(p.T).astype(BF16)           # [128, K]

        iota = np.arange(K)
        for P in range(2):
            ranks = 2 * np.arange(J) + P                       # [J]
            qneg = np.ascontiguousarray(qneg_full[ranks].T)    # [128, J] f32
            # allowed[c, i] = |ang_i - ang_rank_c| < 0.5  and  i < rank_c
            dif = np.abs(ang_s[None, :] - ang_s[ranks][:, None]).astype(F32)
            allowed = (dif < F32(0.5)) & (iota[None, :] < ranks[:, None])
            madd = np.full((TJ, Lsum), MASK_NEG, dtype=F32)
            goff = 0
            for ts, wg in groups:
                off = 0
                for t in ts:
                    L = Ls[t]
                    rows = allowed[TJ * t: TJ * t + TJ, :L]
                    blk = np.where(rows, be2, MASK_NEG)
                    madd[:, goff + off: goff + off + L] = blk
                    off += L
                goff += wg
            in_maps.append({
                "p16": p16,
                "qneg": qneg,
                "qnegT": qneg_full[ranks[:max(TJ * len(_psum_ts(n_t)), 1)]
                                   ].reshape(1, -1).astype(BF16),
                "madd": madd.astype(BF16),
                "we2": we2_16,
                "i8": i8,
                "i128": i128,
                "ones1": ones1,
            })

    res = run_bass_kernel_spmd(nc, in_maps, list(range(N_CORES)))

    for ci in range(N_CORES):
        b, P = ci // 2, ci % 2
        ymat = np.asarray(res.results[ci]["y"], dtype=F32)     # [TJ, n_t]
        nm = np.maximum(ymat.T.reshape(-1), 0.0).astype(F32)   # [J] c=8t+jj
        s1 = np.maximum(nm[:, None] * w["W_n1"][0][None, :] + w["b_n1"],
                        0.0).astype(F32)
        s2 = np.maximum(s1 @ w["W_n2"] + w["b_n2"], 0.0).astype(F32)
        logit = (s2 @ w["W_head"][:, 0] + w["b_head"][0]).astype(F32)
        prob = (1.0 / (1.0 + np.exp(-logit.astype(np.float64)))).astype(F32)
        ranks = 2 * np.arange(J) + P
        valid = ranks < Kbs[b]
        out[b, perms[b][ranks[valid]]] = prob[valid]
    return out
